# revision 2
# baseline (speedup 1.0000x reference)
"""Trainium2 Bass kernel for ConditionedPNA (3-layer PNAConv, N=50000, D=128, DEG=12).

Sharding/strategy (8 NeuronCores, SPMD):
  - Nodes sharded N/8 per core (padded to a multiple of 128). edge_index row is
    sorted (repeat(arange(N), DEG)), so each node's DEG edges are contiguous and
    colocated with the node's core; segment reductions are purely local.
  - Algebra: m_e = A[row_e] + B[col_e] + bpre with A = h @ Wpre[:, :D].T,
    B = h @ Wpre[:, D:].T. All aggregators reduce to segment stats of B[col]:
    mean = C + S/12, max = C + MX, min = C + MN,
    std = sqrt(relu(S2/12 - (S/12)^2) + 1e-5) (the C = A + bpre term cancels).
    deg == DEG everywhere -> degree scalers are constants folded into Wpost;
    Wlin, bpost, blin and the residual are folded in on the host.
  - Per layer: B shard in fp16, pair-packed rows ([NP/2, 2D]) -> AllGather ->
    full fp16 pair table in DRAM (25088 rows < int16 max, the dma_gather index
    limit). Per 128-node tile, TWO dma_gather instructions (768 idxs each,
    <=1024-descriptor/instruction limit) fetch the 12 neighbor pair-rows per
    node; a per-edge parity mask selects the even/odd half (arith select in
    fp16). This replaces 12 indirect DMAs/tile (994ns fixed Pool-engine
    descriptor-gen cost per DMA instruction was the bottleneck).
  - Reduction trees S/MX/MN/S2 run in fp16 with f32 final level; agg chain,
    PE transposes of the four agg parts, and the 5-matmul post-GEMM
    (h'^T = WhhT.T@h^T + sum_j WeffT_j.T@part_j^T + bias, residual folded
    into Whh) stay f32.
"""
import math
import numpy as np

import concourse.bass as bass
from concourse import bacc
import concourse.tile as tile
from concourse import mybir
from concourse.masks import make_identity
from concourse.bass_utils import run_bass_kernel_spmd

D, DEG, L, CORES = 128, 12, 3, 8
NQ = 4  # SWDGE queues
F32 = mybir.dt.float32
F16 = mybir.dt.float16
I16 = mybir.dt.int16

_hist = np.array([1.0] * 10 + [2.0] * 10)
AVG_LOG = float((np.log(np.arange(20) + 1.0) * _hist).sum() / _hist.sum())


class Cfg:
    def __init__(self, n):
        self.N = n
        self.NC = n // CORES
        self.NP = ((self.NC + 127) // 128) * 128
        self.TILES = self.NP // 128
        self.AG_ROWS = CORES * self.NP


def _tree(nc, eng, g3, work, out, opname):
    """g3: (128, 12, 128) f16 AP; work: (128, 6, 128) f16; out f32 (128,128).
    Levels 1-3 in fp16 (2x DVE), final level converts to f32."""
    A = mybir.AluOpType
    op = {"add": A.add, "max": A.max, "min": A.min}[opname]
    eng.tensor_tensor(out=work[:, 0:6, :], in0=g3[:, 0:6, :], in1=g3[:, 6:12, :], op=op)
    eng.tensor_tensor(out=work[:, 0:3, :], in0=work[:, 0:3, :], in1=work[:, 3:6, :], op=op)
    eng.tensor_tensor(out=work[:, 0:1, :], in0=work[:, 0:1, :], in1=work[:, 2:3, :], op=op)
    eng.tensor_tensor(out=out[:], in0=work[:, 0, :], in1=work[:, 1, :], op=op)


def build(cfg, repeat=1, ablate="FULL"):
    A = mybir.AluOpType
    AF = mybir.ActivationFunctionType
    NP, TILES = cfg.NP, cfg.TILES
    NPAIR = cfg.AG_ROWS // 2
    nc = bacc.Bacc("TRN2", target_bir_lowering=False, num_devices=CORES,
                   num_swdge_queues=NQ)

    xT = nc.dram_tensor("xT", [D, NP], F32, kind="ExternalInput")
    idx = nc.dram_tensor("idx", [TILES, 128, 96], I16, kind="ExternalInput")
    msk = nc.dram_tensor("msk", [TILES, 128, DEG], F16, kind="ExternalInput")
    wpack = nc.dram_tensor("wpack", [D, L, 7, D], F32, kind="ExternalInput")
    bpack = nc.dram_tensor("bpack", [D, L, 2], F32, kind="ExternalInput")
    outT = nc.dram_tensor("outT", [D, NP], F32, kind="ExternalOutput")

    agin = nc.dram_tensor("agin", [NP // 2, 2 * D], F16)
    agout = nc.dram_tensor("agout", [NPAIR, 2 * D], F16, addr_space="Shared")

    with tile.TileContext(nc) as tc:
        with (
            tc.tile_pool(name="persist", bufs=1) as pp,
            tc.tile_pool(name="gat", bufs=3) as gp,
            tc.tile_pool(name="work", bufs=2) as wp,
            tc.tile_pool(name="small", bufs=2) as sp,
            tc.tile_pool(name="psum", bufs=2, space="PSUM") as ps,
        ):
            hT_a = pp.tile([D, NP], F32)
            hT_b = pp.tile([D, NP], F32)
            hT = [hT_a, hT_b]
            W = pp.tile([D, L, 7, D], F32)
            nc.sync.dma_start(out=W[:], in_=wpack[:])
            B = pp.tile([D, L, 2], F32)
            nc.sync.dma_start(out=B[:], in_=bpack[:])
            eps = pp.tile([D, 1], F32)
            nc.vector.memset(eps[:], 1e-5)
            ident = pp.tile([D, D], F32)
            make_identity(nc, ident[:])
            nc.sync.dma_start(out=hT[0][:], in_=xT[:])

            for ll in range(repeat * L):
                l = ll % L
                hin = hT[ll % 2]
                hout = hT[(ll + 1) % 2]
                w1t = W[:, l, 0, :]
                w2t = W[:, l, 1, :]
                whh = W[:, l, 2, :]
                weff = [W[:, l, 3 + j, :] for j in range(4)]
                bout = B[:, l, 1:2]

                # ---- B shard (node-major, fp16 pair rows) -> agin -> AllGather
                for t in range(TILES):
                    bp = ps.tile([128, D], F32, space="PSUM", tag="bp")
                    nc.tensor.matmul(
                        out=bp[:], lhsT=hin[:, t * 128:(t + 1) * 128], rhs=w2t,
                        start=True, stop=True,
                    )
                    bs = sp.tile([128, D], F16, tag="bs")
                    nc.scalar.activation(bs[:], bp[:], AF.Copy)
                    nc.sync.dma_start(
                        out=agin[t * 64:(t + 1) * 64, :], in_=bs[:])
                nc.gpsimd.collective_compute(
                    "AllGather", A.bypass,
                    replica_groups=[list(range(CORES))],
                    ins=[agin[:]], outs=[agout[:]],
                )

                # ---- per 128-node tile ----
                for t in range(TILES):
                    hslab = hin[:, t * 128:(t + 1) * 128]
                    itile = sp.tile([128, 96], I16, tag="itile")
                    nc.sync.dma_start(out=itile[:], in_=idx[t])
                    mt = sp.tile([128, DEG], F16, tag="mt")
                    nc.sync.dma_start(out=mt[:], in_=msk[t])
                    pairt = gp.tile([128, DEG, 2 * D], F16, tag="pair")
                    for g in range(2):
                        nc.gpsimd.dma_gather(
                            out_ap=pairt[:, 6 * g:6 * g + 6, :], in_ap=agout[:],
                            idxs_ap=itile[:, 48 * g:48 * g + 48],
                            num_idxs=768, num_idxs_reg=768, elem_size=2 * D,
                            queue_num=(2 * t + g) % NQ,
                        )

                    # C = A (node-major); bpre is folded into bout on the host
                    apsum = ps.tile([128, D], F32, space="PSUM", tag="apsum")
                    nc.tensor.matmul(out=apsum[:], lhsT=hslab, rhs=w1t, start=True, stop=True)
                    C = sp.tile([128, D], F32, tag="C")
                    nc.scalar.activation(C[:], apsum[:], AF.Copy)

                    # select even/odd pair half per edge: gmh = even + mask*(odd-even)
                    gmh = gp.tile([128, DEG, D], F16, tag="gmh")
                    ev = pairt[:, :, 0:D]
                    od = pairt[:, :, D:2 * D]
                    mb3 = mt[:].unsqueeze(2).broadcast_to([128, DEG, D])
                    nc.vector.tensor_tensor(out=gmh[:], in0=od, in1=ev, op=A.subtract)
                    nc.vector.tensor_tensor(out=gmh[:], in0=gmh[:], in1=mb3, op=A.mult)
                    nc.vector.tensor_tensor(out=gmh[:], in0=gmh[:], in1=ev, op=A.add)

                    g2 = gp.tile([128, DEG, D], F16, tag="g2")
                    nc.scalar.square(g2[:], gmh[:])
                    wS = wp.tile([128, 6, D], F16, tag="wS")
                    wX = wp.tile([128, 6, D], F16, tag="wX")
                    wN = wp.tile([128, 6, D], F16, tag="wN")
                    w2b = wp.tile([128, 6, D], F16, tag="w2b")
                    S = sp.tile([128, D], F32, tag="S")
                    MX = sp.tile([128, D], F32, tag="MX")
                    MN = sp.tile([128, D], F32, tag="MN")
                    S2 = sp.tile([128, D], F32, tag="S2")
                    _tree(nc, nc.vector, gmh[:], wS, S[:], "add")
                    _tree(nc, nc.vector, gmh[:], wX, MX[:], "max")
                    _tree(nc, nc.vector, gmh[:], wN, MN[:], "min")
                    _tree(nc, nc.vector, g2[:], w2b, S2[:], "add")

                    meanB = sp.tile([128, D], F32, tag="meanB")
                    nc.vector.tensor_scalar_mul(meanB[:], S[:], 1.0 / DEG)
                    mean = sp.tile([128, D], F32, tag="mean")
                    nc.vector.tensor_tensor(out=mean[:], in0=meanB[:], in1=C[:], op=A.add)
                    mxc = sp.tile([128, D], F32, tag="mxc")
                    nc.vector.tensor_tensor(out=mxc[:], in0=MX[:], in1=C[:], op=A.add)
                    mnc = sp.tile([128, D], F32, tag="mnc")
                    nc.vector.tensor_tensor(out=mnc[:], in0=MN[:], in1=C[:], op=A.add)
                    m2 = sp.tile([128, D], F32, tag="m2")
                    nc.vector.tensor_tensor(out=m2[:], in0=meanB[:], in1=meanB[:], op=A.mult)
                    var = sp.tile([128, D], F32, tag="var")
                    nc.vector.scalar_tensor_tensor(
                        out=var[:], in0=S2[:], scalar=1.0 / DEG, in1=m2[:],
                        op0=A.mult, op1=A.subtract,
                    )
                    varc = sp.tile([128, D], F32, tag="varc")
                    nc.vector.tensor_scalar_max(varc[:], var[:], 0.0)
                    std = sp.tile([128, D], F32, tag="std")
                    nc.scalar.activation(std[:], varc[:], AF.Sqrt, bias=eps[:])

                    # transpose agg parts to feature-major and post-GEMM
                    hp = ps.tile([128, 128], F32, space="PSUM", tag="hp")
                    nc.tensor.matmul(out=hp[:], lhsT=whh, rhs=hslab, start=True, stop=False)
                    for j, part in enumerate([mean, mxc, mnc, std]):
                        ptp = ps.tile([128, 128], F32, space="PSUM", tag="tp")
                        nc.tensor.transpose(out=ptp[:], in_=part[:], identity=ident[:])
                        pts = sp.tile([128, 128], F32, tag="pts")
                        if j % 2 == 0:
                            nc.scalar.activation(pts[:], ptp[:], AF.Copy)
                        else:
                            nc.vector.tensor_copy(pts[:], ptp[:])
                        nc.tensor.matmul(
                            out=hp[:], lhsT=weff[j], rhs=pts[:],
                            start=False, stop=(j == 3),
                        )
                    nc.scalar.activation(
                        hout[:, t * 128:(t + 1) * 128], hp[:], AF.Identity, bias=bout,
                    )
                if cfg.NC < NP:
                    nc.vector.memset(hout[:, cfg.NC:NP], 0.0)

            nc.sync.dma_start(out=outT[:], in_=hT[(repeat * L) % 2][:])
    nc.compile()
    return nc


def prep_inputs(cfg, x, edge_index, Wpre, bpre, Wpost, bpost, Wlin, blin):
    x = np.asarray(x, np.float32)
    ei = np.asarray(edge_index)
    Wpre = np.asarray(Wpre, np.float32)
    bpre = np.asarray(bpre, np.float32)
    Wpost = np.asarray(Wpost, np.float32)
    bpost = np.asarray(bpost, np.float32)
    Wlin = np.asarray(Wlin, np.float32)
    blin = np.asarray(blin, np.float32)
    N, NC, NP, TILES = cfg.N, cfg.NC, cfg.NP, cfg.TILES

    row = ei[0].astype(np.int64)
    col = ei[1].astype(np.int64)
    assert (row == np.repeat(np.arange(N), DEG)).all(), "kernel assumes sorted rows, uniform degree"
    dlog = math.log(DEG + 1.0)
    k1 = dlog / AVG_LOG
    k2 = AVG_LOG / dlog

    wpack = np.zeros((D, L, 7, D), np.float32)
    bpack = np.zeros((D, L, 2), np.float32)
    I = np.eye(D, dtype=np.float32)
    for l in range(L):
        W1 = Wpre[l][:, :D]
        W2 = Wpre[l][:, D:]
        Wh = Wpost[l][:, :D]
        Wid = Wpost[l][:, D:5 * D]
        Wamp = Wpost[l][:, 5 * D:9 * D]
        Watt = Wpost[l][:, 9 * D:13 * D]
        Weff = Wlin[l] @ (Wid + k1 * Wamp + k2 * Watt)
        Whh = Wlin[l] @ Wh + I
        wpack[:, l, 0, :] = W1.T
        wpack[:, l, 1, :] = W2.T
        wpack[:, l, 2, :] = Whh.T
        for j in range(4):
            wpack[:, l, 3 + j, :] = Weff[:, j * D:(j + 1) * D].T
        bpack[:, l, 0] = bpre[l]
        bpack[:, l, 1] = (Wlin[l] @ bpost[l] + blin[l]
                          + (Weff[:, :D] + Weff[:, D:2*D] + Weff[:, 2*D:3*D]) @ bpre[l])

    in_maps = []
    for c in range(CORES):
        xs = x[c * NC:(c + 1) * NC]
        xT = np.zeros((D, NP), np.float32)
        xT[:, :NC] = xs.T
        cols = col[c * NC * DEG:(c + 1) * NC * DEG]
        cols = np.concatenate([cols, np.zeros(((NP - NC) * DEG,), np.int64)])
        cols = cols.reshape(NP, DEG)
        gr = (cols // NC) * NP + (cols % NC)  # global row in the AllGather table
        pr = (gr // 2).astype(np.int16).reshape(TILES, 128, DEG)
        par = (gr % 2).astype(np.float16).reshape(TILES, 128, DEG)
        # dma_gather idx layout: 2 gathers of 768 (slots 0-5 / 6-11); linear
        # position i = k_local*128 + p lives at [i%16, i//16], and the 16-row
        # block is replicated across all 8 GPSIMD cores' partition windows.
        idxa = np.zeros((TILES, 128, 96), np.int16)
        for g in range(2):
            sub = pr[:, :, 6 * g:6 * g + 6]             # (T, 128, 6)
            v = sub.transpose(0, 2, 1).reshape(TILES, 768)
            block = v.reshape(TILES, 48, 16).transpose(0, 2, 1)  # (T, 16, 48)
            idxa[:, :, 48 * g:48 * g + 48] = np.tile(block, (1, 8, 1))
        in_maps.append({
            "xT": xT,
            "idx": idxa,
            "msk": par,
            "wpack": wpack,
            "bpack": bpack,
        })
    return in_maps


_CACHE = {}


def kernel(x, edge_index, Wpre, bpre, Wpost, bpost, Wlin, blin):
    cfg = Cfg(np.asarray(x).shape[0])
    in_maps = prep_inputs(cfg, x, edge_index, Wpre, bpre, Wpost, bpost, Wlin, blin)
    if cfg.N not in _CACHE:
        _CACHE[cfg.N] = build(cfg)
    nc = _CACHE[cfg.N]
    res = run_bass_kernel_spmd(nc, in_maps, list(range(CORES)))
    outs = []
    for c in range(CORES):
        oT = res.results[c]["outT"]
        outs.append(np.ascontiguousarray(oT[:, :cfg.NC].T))
    return np.concatenate(outs, axis=0).astype(np.float32)


# revision 4
# speedup vs baseline: 1.0287x; 1.0287x over previous
"""Trainium2 Bass kernel for ConditionedPNA (3-layer PNAConv, N=50000, D=128, DEG=12).

Sharding/strategy (8 NeuronCores, SPMD):
  - Nodes sharded N/8 per core (padded to a multiple of 128). edge_index row is
    sorted (repeat(arange(N), DEG)), so each node's DEG edges are contiguous and
    colocated with the node's core; segment reductions are purely local.
  - Algebra: m_e = A[row_e] + B[col_e] + bpre with A = h @ Wpre[:, :D].T,
    B = h @ Wpre[:, D:].T. All aggregators reduce to segment stats of B[col]:
    mean = C + S/12, max = C + MX, min = C + MN,
    std = sqrt(relu(S2/12 - (S/12)^2) + 1e-5) (the C = A + bpre term cancels).
    deg == DEG everywhere -> degree scalers are constants folded into Wpost;
    Wlin, bpost, blin and the residual are folded in on the host.
  - Per layer: B shard in fp16, pair-packed rows ([NP/2, 2D]) -> AllGather ->
    full fp16 pair table in DRAM (25088 rows < int16 max, the dma_gather index
    limit). Per 128-node tile, TWO dma_gather instructions (768 idxs each,
    <=1024-descriptor/instruction limit) fetch the 12 neighbor pair-rows per
    node; a per-edge parity mask selects the even/odd half (arith select in
    fp16). This replaces 12 indirect DMAs/tile (994ns fixed Pool-engine
    descriptor-gen cost per DMA instruction was the bottleneck).
  - Reduction trees S/MX/MN/S2 run in fp16 with f32 final level; agg chain,
    PE transposes of the four agg parts, and the 5-matmul post-GEMM
    (h'^T = WhhT.T@h^T + sum_j WeffT_j.T@part_j^T + bias, residual folded
    into Whh) stay f32.
"""
import math
import numpy as np

import concourse.bass as bass
from concourse import bacc
import concourse.tile as tile
from concourse import mybir
from concourse.masks import make_identity
from concourse.bass_utils import run_bass_kernel_spmd

D, DEG, L, CORES = 128, 12, 3, 8
NQ = 4  # SWDGE queues
F32 = mybir.dt.float32
F16 = mybir.dt.float16
I16 = mybir.dt.int16

_hist = np.array([1.0] * 10 + [2.0] * 10)
AVG_LOG = float((np.log(np.arange(20) + 1.0) * _hist).sum() / _hist.sum())


class Cfg:
    def __init__(self, n):
        self.N = n
        self.NC = n // CORES
        self.NP = ((self.NC + 127) // 128) * 128
        self.TILES = self.NP // 128
        self.AG_ROWS = CORES * self.NP


def _tree(nc, eng, g3, work, out, opname):
    """g3: (128, 12, 128) f16 AP; work: (128, 6, 128) f16; out f32 (128,128).
    Levels 1-3 in fp16 (2x DVE), final level converts to f32."""
    A = mybir.AluOpType
    op = {"add": A.add, "max": A.max, "min": A.min}[opname]
    eng.tensor_tensor(out=work[:, 0:6, :], in0=g3[:, 0:6, :], in1=g3[:, 6:12, :], op=op)
    eng.tensor_tensor(out=work[:, 0:3, :], in0=work[:, 0:3, :], in1=work[:, 3:6, :], op=op)
    eng.tensor_tensor(out=work[:, 0:1, :], in0=work[:, 0:1, :], in1=work[:, 2:3, :], op=op)
    eng.tensor_tensor(out=out[:], in0=work[:, 0, :], in1=work[:, 1, :], op=op)


def build(cfg, repeat=1, ablate="FULL"):
    A = mybir.AluOpType
    AF = mybir.ActivationFunctionType
    NP, TILES = cfg.NP, cfg.TILES
    NPAIR = cfg.AG_ROWS // 2
    nc = bacc.Bacc("TRN2", target_bir_lowering=False, num_devices=CORES,
                   num_swdge_queues=NQ)

    xT = nc.dram_tensor("xT", [D, NP], F16, kind="ExternalInput")
    idx = nc.dram_tensor("idx", [TILES, 128, 96], I16, kind="ExternalInput")
    msk = nc.dram_tensor("msk", [TILES, 128, DEG], F16, kind="ExternalInput")
    wpack = nc.dram_tensor("wpack", [D, L, 7, D], F16, kind="ExternalInput")
    bpack = nc.dram_tensor("bpack", [D, L, 2], F32, kind="ExternalInput")
    outT = nc.dram_tensor("outT", [D, NP], F16, kind="ExternalOutput")

    agin = nc.dram_tensor("agin", [NP // 2, 2 * D], F16)
    agout = nc.dram_tensor("agout", [NPAIR, 2 * D], F16, addr_space="Shared")

    with tile.TileContext(nc) as tc:
        with (
            tc.tile_pool(name="persist", bufs=1) as pp,
            tc.tile_pool(name="gat", bufs=3) as gp,
            tc.tile_pool(name="work", bufs=2) as wp,
            tc.tile_pool(name="small", bufs=2) as sp,
            tc.tile_pool(name="psum", bufs=2, space="PSUM") as ps,
        ):
            hT_a = pp.tile([D, NP], F16)
            hT_b = pp.tile([D, NP], F16)
            hT = [hT_a, hT_b]
            W = pp.tile([D, L, 7, D], F16)
            nc.sync.dma_start(out=W[:], in_=wpack[:])
            B = pp.tile([D, L, 2], F32)
            nc.sync.dma_start(out=B[:], in_=bpack[:])
            eps = pp.tile([D, 1], F32)
            nc.vector.memset(eps[:], 1e-5)
            ident = pp.tile([D, D], F32)
            make_identity(nc, ident[:])
            nc.sync.dma_start(out=hT[0][:], in_=xT[:])

            for ll in range(repeat * L):
                l = ll % L
                hin = hT[ll % 2]
                hout = hT[(ll + 1) % 2]
                w1t = W[:, l, 0, :]
                w2t = W[:, l, 1, :]
                whh = W[:, l, 2, :]
                weff = [W[:, l, 3 + j, :] for j in range(4)]
                bout = B[:, l, 1:2]

                # ---- B shard (node-major, fp16 pair rows) -> agin -> AllGather
                for t in range(TILES):
                    bp = ps.tile([128, D], F32, space="PSUM", tag="bp")
                    nc.tensor.matmul(
                        out=bp[:], lhsT=hin[:, t * 128:(t + 1) * 128], rhs=w2t,
                        start=True, stop=True,
                    )
                    bs = sp.tile([128, D], F16, tag="bs")
                    nc.scalar.activation(bs[:], bp[:], AF.Copy)
                    nc.sync.dma_start(
                        out=agin[t * 64:(t + 1) * 64, :], in_=bs[:])
                nc.gpsimd.collective_compute(
                    "AllGather", A.bypass,
                    replica_groups=[list(range(CORES))],
                    ins=[agin[:]], outs=[agout[:]],
                )

                # ---- per 128-node tile ----
                for t in range(TILES):
                    hslab = hin[:, t * 128:(t + 1) * 128]
                    itile = sp.tile([128, 96], I16, tag="itile")
                    nc.sync.dma_start(out=itile[:], in_=idx[t])
                    mt = sp.tile([128, DEG], F16, tag="mt")
                    nc.sync.dma_start(out=mt[:], in_=msk[t])
                    pairt = gp.tile([128, DEG, 2 * D], F16, tag="pair")
                    for g in range(2):
                        nc.gpsimd.dma_gather(
                            out_ap=pairt[:, 6 * g:6 * g + 6, :], in_ap=agout[:],
                            idxs_ap=itile[:, 48 * g:48 * g + 48],
                            num_idxs=768, num_idxs_reg=768, elem_size=2 * D,
                            queue_num=(2 * t + g) % NQ,
                        )

                    # C = A (node-major); bpre is folded into bout on the host
                    apsum = ps.tile([128, D], F32, space="PSUM", tag="apsum")
                    nc.tensor.matmul(out=apsum[:], lhsT=hslab, rhs=w1t, start=True, stop=True)
                    C = sp.tile([128, D], F32, tag="C")
                    nc.scalar.activation(C[:], apsum[:], AF.Copy)

                    # select even/odd pair half per edge: gmh = even + mask*(odd-even)
                    gmh = gp.tile([128, DEG, D], F16, tag="gmh")
                    ev = pairt[:, :, 0:D]
                    od = pairt[:, :, D:2 * D]
                    mb3 = mt[:].unsqueeze(2).broadcast_to([128, DEG, D])
                    nc.vector.tensor_tensor(out=gmh[:], in0=od, in1=ev, op=A.subtract)
                    nc.vector.tensor_tensor(out=gmh[:], in0=gmh[:], in1=mb3, op=A.mult)
                    nc.vector.tensor_tensor(out=gmh[:], in0=gmh[:], in1=ev, op=A.add)

                    g2 = gp.tile([128, DEG, D], F16, tag="g2")
                    nc.scalar.square(g2[:], gmh[:])
                    wS = wp.tile([128, 6, D], F16, tag="wS")
                    wX = wp.tile([128, 6, D], F16, tag="wX")
                    wN = wp.tile([128, 6, D], F16, tag="wN")
                    w2b = wp.tile([128, 6, D], F16, tag="w2b")
                    S = sp.tile([128, D], F32, tag="S")
                    MX = sp.tile([128, D], F32, tag="MX")
                    MN = sp.tile([128, D], F32, tag="MN")
                    S2 = sp.tile([128, D], F32, tag="S2")
                    _tree(nc, nc.vector, gmh[:], wS, S[:], "add")
                    _tree(nc, nc.vector, gmh[:], wX, MX[:], "max")
                    _tree(nc, nc.vector, gmh[:], wN, MN[:], "min")
                    _tree(nc, nc.vector, g2[:], w2b, S2[:], "add")

                    meanB = sp.tile([128, D], F32, tag="meanB")
                    nc.vector.tensor_scalar_mul(meanB[:], S[:], 1.0 / DEG)
                    mean = sp.tile([128, D], F32, tag="mean")
                    nc.vector.tensor_tensor(out=mean[:], in0=meanB[:], in1=C[:], op=A.add)
                    mxc = sp.tile([128, D], F32, tag="mxc")
                    nc.vector.tensor_tensor(out=mxc[:], in0=MX[:], in1=C[:], op=A.add)
                    mnc = sp.tile([128, D], F32, tag="mnc")
                    nc.vector.tensor_tensor(out=mnc[:], in0=MN[:], in1=C[:], op=A.add)
                    m2 = sp.tile([128, D], F32, tag="m2")
                    nc.vector.tensor_tensor(out=m2[:], in0=meanB[:], in1=meanB[:], op=A.mult)
                    var = sp.tile([128, D], F32, tag="var")
                    nc.vector.scalar_tensor_tensor(
                        out=var[:], in0=S2[:], scalar=1.0 / DEG, in1=m2[:],
                        op0=A.mult, op1=A.subtract,
                    )
                    varc = sp.tile([128, D], F32, tag="varc")
                    nc.vector.tensor_scalar_max(varc[:], var[:], 0.0)
                    std = sp.tile([128, D], F32, tag="std")
                    nc.scalar.activation(std[:], varc[:], AF.Sqrt, bias=eps[:])

                    # transpose agg parts to feature-major and post-GEMM
                    hp = ps.tile([128, 128], F32, space="PSUM", tag="hp")
                    nc.tensor.matmul(out=hp[:], lhsT=whh, rhs=hslab, start=True, stop=False)
                    for j, part in enumerate([mean, mxc, mnc, std]):
                        ptp = ps.tile([128, 128], F32, space="PSUM", tag="tp")
                        nc.tensor.transpose(out=ptp[:], in_=part[:], identity=ident[:])
                        pts = sp.tile([128, 128], F16, tag="pts")
                        if j % 2 == 0:
                            nc.scalar.activation(pts[:], ptp[:], AF.Copy)
                        else:
                            nc.vector.tensor_copy(pts[:], ptp[:])
                        nc.tensor.matmul(
                            out=hp[:], lhsT=weff[j], rhs=pts[:],
                            start=False, stop=(j == 3),
                        )
                    nc.scalar.activation(
                        hout[:, t * 128:(t + 1) * 128], hp[:], AF.Identity, bias=bout,
                    )
                if cfg.NC < NP:
                    nc.vector.memset(hout[:, cfg.NC:NP], 0.0)

            nc.sync.dma_start(out=outT[:], in_=hT[(repeat * L) % 2][:])
    nc.compile()
    return nc


def prep_inputs(cfg, x, edge_index, Wpre, bpre, Wpost, bpost, Wlin, blin):
    x = np.asarray(x, np.float32)
    ei = np.asarray(edge_index)
    Wpre = np.asarray(Wpre, np.float32)
    bpre = np.asarray(bpre, np.float32)
    Wpost = np.asarray(Wpost, np.float32)
    bpost = np.asarray(bpost, np.float32)
    Wlin = np.asarray(Wlin, np.float32)
    blin = np.asarray(blin, np.float32)
    N, NC, NP, TILES = cfg.N, cfg.NC, cfg.NP, cfg.TILES

    row = ei[0].astype(np.int64)
    col = ei[1].astype(np.int64)
    assert (row == np.repeat(np.arange(N), DEG)).all(), "kernel assumes sorted rows, uniform degree"
    dlog = math.log(DEG + 1.0)
    k1 = dlog / AVG_LOG
    k2 = AVG_LOG / dlog

    wpack = np.zeros((D, L, 7, D), np.float16)
    bpack = np.zeros((D, L, 2), np.float32)
    I = np.eye(D, dtype=np.float32)
    for l in range(L):
        W1 = Wpre[l][:, :D]
        W2 = Wpre[l][:, D:]
        Wh = Wpost[l][:, :D]
        Wid = Wpost[l][:, D:5 * D]
        Wamp = Wpost[l][:, 5 * D:9 * D]
        Watt = Wpost[l][:, 9 * D:13 * D]
        Weff = Wlin[l] @ (Wid + k1 * Wamp + k2 * Watt)
        Whh = Wlin[l] @ Wh + I
        wpack[:, l, 0, :] = W1.T
        wpack[:, l, 1, :] = W2.T
        wpack[:, l, 2, :] = Whh.T
        for j in range(4):
            wpack[:, l, 3 + j, :] = Weff[:, j * D:(j + 1) * D].T
        bpack[:, l, 0] = bpre[l]
        bpack[:, l, 1] = (Wlin[l] @ bpost[l] + blin[l]
                          + (Weff[:, :D] + Weff[:, D:2*D] + Weff[:, 2*D:3*D]) @ bpre[l])

    in_maps = []
    for c in range(CORES):
        xs = x[c * NC:(c + 1) * NC]
        xT = np.zeros((D, NP), np.float16)
        xT[:, :NC] = xs.T
        cols = col[c * NC * DEG:(c + 1) * NC * DEG]
        cols = np.concatenate([cols, np.zeros(((NP - NC) * DEG,), np.int64)])
        cols = cols.reshape(NP, DEG)
        gr = (cols // NC) * NP + (cols % NC)  # global row in the AllGather table
        pr = (gr // 2).astype(np.int16).reshape(TILES, 128, DEG)
        par = (gr % 2).astype(np.float16).reshape(TILES, 128, DEG)
        # dma_gather idx layout: 2 gathers of 768 (slots 0-5 / 6-11); linear
        # position i = k_local*128 + p lives at [i%16, i//16], and the 16-row
        # block is replicated across all 8 GPSIMD cores' partition windows.
        idxa = np.zeros((TILES, 128, 96), np.int16)
        for g in range(2):
            sub = pr[:, :, 6 * g:6 * g + 6]             # (T, 128, 6)
            v = sub.transpose(0, 2, 1).reshape(TILES, 768)
            block = v.reshape(TILES, 48, 16).transpose(0, 2, 1)  # (T, 16, 48)
            idxa[:, :, 48 * g:48 * g + 48] = np.tile(block, (1, 8, 1))
        in_maps.append({
            "xT": xT,
            "idx": idxa,
            "msk": par,
            "wpack": wpack,
            "bpack": bpack,
        })
    return in_maps


_CACHE = {}


def kernel(x, edge_index, Wpre, bpre, Wpost, bpost, Wlin, blin):
    cfg = Cfg(np.asarray(x).shape[0])
    in_maps = prep_inputs(cfg, x, edge_index, Wpre, bpre, Wpost, bpost, Wlin, blin)
    if cfg.N not in _CACHE:
        _CACHE[cfg.N] = build(cfg)
    nc = _CACHE[cfg.N]
    res = run_bass_kernel_spmd(nc, in_maps, list(range(CORES)))
    outs = []
    for c in range(CORES):
        oT = res.results[c]["outT"]
        outs.append(np.ascontiguousarray(oT[:, :cfg.NC].T))
    return np.concatenate(outs, axis=0).astype(np.float32)


# revision 5
# speedup vs baseline: 6.2907x; 6.1152x over previous
"""Trainium2 Bass kernel for ConditionedPNA (3-layer PNAConv, N=50000, D=128, DEG=12).

Sharding/strategy (8 NeuronCores, SPMD):
  - Nodes sharded N/8 per core (padded to a multiple of 128). edge_index row is
    sorted (repeat(arange(N), DEG)), so each node's DEG edges are contiguous and
    colocated with the node's core; segment reductions are purely local.
  - Algebra: m_e = A[row_e] + B[col_e] + bpre with A = h @ Wpre[:, :D].T,
    B = h @ Wpre[:, D:].T. All aggregators reduce to segment stats of B[col]:
    mean = C + S/12, max = C + MX, min = C + MN,
    std = sqrt(relu(S2/12 - (S/12)^2) + 1e-5) (the C = A + bpre term cancels).
    deg == DEG everywhere -> degree scalers are constants folded into Wpost;
    Wlin, bpost, blin and the residual are folded in on the host.
  - Per layer: B shard in fp16, pair-packed rows ([NP/2, 2D]) -> AllGather ->
    full fp16 pair table in DRAM (25088 rows < int16 max, the dma_gather index
    limit). Per 128-node tile, TWO dma_gather instructions (768 idxs each,
    <=1024-descriptor/instruction limit) fetch the 12 neighbor pair-rows per
    node; a per-edge parity mask selects the even/odd half (arith select in
    fp16). This replaces 12 indirect DMAs/tile (994ns fixed Pool-engine
    descriptor-gen cost per DMA instruction was the bottleneck).
  - Reduction trees S/MX/MN/S2 run in fp16 with f32 final level; agg chain,
    PE transposes of the four agg parts, and the 5-matmul post-GEMM
    (h'^T = WhhT.T@h^T + sum_j WeffT_j.T@part_j^T + bias, residual folded
    into Whh) stay f32.
"""
import math
import numpy as np

import concourse.bass as bass
from concourse import bacc
import concourse.tile as tile
from concourse import mybir
from concourse.masks import make_identity
from concourse.bass_utils import run_bass_kernel_spmd

D, DEG, L, CORES = 128, 12, 3, 8
NQ = 4  # SWDGE queues
F32 = mybir.dt.float32
F16 = mybir.dt.float16
I16 = mybir.dt.int16

_hist = np.array([1.0] * 10 + [2.0] * 10)
AVG_LOG = float((np.log(np.arange(20) + 1.0) * _hist).sum() / _hist.sum())


class Cfg:
    def __init__(self, n):
        self.N = n
        self.NC = n // CORES
        self.NP = ((self.NC + 127) // 128) * 128
        self.TILES = self.NP // 128
        self.AG_ROWS = CORES * self.NP


def _tree(nc, eng, g3, work, out, opname):
    """g3: (128, 12, 128) f16 AP; work: (128, 6, 128) f16; out f32 (128,128).
    Levels 1-3 in fp16 (2x DVE), final level converts to f32."""
    A = mybir.AluOpType
    op = {"add": A.add, "max": A.max, "min": A.min}[opname]
    eng.tensor_tensor(out=work[:, 0:6, :], in0=g3[:, 0:6, :], in1=g3[:, 6:12, :], op=op)
    eng.tensor_tensor(out=work[:, 0:3, :], in0=work[:, 0:3, :], in1=work[:, 3:6, :], op=op)
    eng.tensor_tensor(out=work[:, 0:1, :], in0=work[:, 0:1, :], in1=work[:, 2:3, :], op=op)
    eng.tensor_tensor(out=out[:], in0=work[:, 0, :], in1=work[:, 1, :], op=op)


def build(cfg, repeat=1, ablate="FULL"):
    A = mybir.AluOpType
    AF = mybir.ActivationFunctionType
    NP, TILES = cfg.NP, cfg.TILES
    NPAIR = cfg.AG_ROWS // 2
    nc = bacc.Bacc("TRN2", target_bir_lowering=False, num_devices=CORES,
                   num_swdge_queues=NQ)

    xT = nc.dram_tensor("xT", [D, NP], F32, kind="ExternalInput")
    idx = nc.dram_tensor("idx", [TILES, 128, 96], I16, kind="ExternalInput")
    msk = nc.dram_tensor("msk", [TILES, 128, DEG], F16, kind="ExternalInput")
    wpack = nc.dram_tensor("wpack", [D, L, 7, D], F32, kind="ExternalInput")
    bpack = nc.dram_tensor("bpack", [D, L, 2], F32, kind="ExternalInput")
    outT = nc.dram_tensor("outT", [D, NP], F32, kind="ExternalOutput")

    agin = nc.dram_tensor("agin", [NP // 2, 2 * D], F16)
    agout = nc.dram_tensor("agout", [NPAIR, 2 * D], F16, addr_space="Shared")

    with tile.TileContext(nc) as tc:
        with (
            tc.tile_pool(name="persist", bufs=1) as pp,
            tc.tile_pool(name="gat", bufs=3) as gp,
            tc.tile_pool(name="work", bufs=2) as wp,
            tc.tile_pool(name="small", bufs=2) as sp,
            tc.tile_pool(name="psum", bufs=2, space="PSUM") as ps,
        ):
            hT_a = pp.tile([D, NP], F32)
            hT_b = pp.tile([D, NP], F32)
            hT = [hT_a, hT_b]
            W = pp.tile([D, L, 7, D], F32)
            nc.sync.dma_start(out=W[:], in_=wpack[:])
            B = pp.tile([D, L, 2], F32)
            nc.sync.dma_start(out=B[:], in_=bpack[:])
            eps = pp.tile([D, 1], F32)
            nc.vector.memset(eps[:], 1e-5)
            ident = pp.tile([D, D], F32)
            make_identity(nc, ident[:])
            nc.sync.dma_start(out=hT[0][:], in_=xT[:])

            for ll in range(repeat * L):
                l = ll % L
                hin = hT[ll % 2]
                hout = hT[(ll + 1) % 2]
                w1t = W[:, l, 0, :]
                w2t = W[:, l, 1, :]
                whh = W[:, l, 2, :]
                weff = [W[:, l, 3 + j, :] for j in range(4)]
                bout = B[:, l, 1:2]

                # ---- B shard (node-major, fp16 pair rows) -> agin -> AllGather
                for t in range(TILES):
                    bp = ps.tile([128, D], F32, space="PSUM", tag="bp")
                    nc.tensor.matmul(
                        out=bp[:], lhsT=hin[:, t * 128:(t + 1) * 128], rhs=w2t,
                        start=True, stop=True,
                    )
                    bs = sp.tile([128, D], F16, tag="bs")
                    nc.scalar.activation(bs[:], bp[:], AF.Copy)
                    nc.sync.dma_start(
                        out=agin[t * 64:(t + 1) * 64, :], in_=bs[:])
                nc.gpsimd.collective_compute(
                    "AllGather", A.bypass,
                    replica_groups=[list(range(CORES))],
                    ins=[agin[:]], outs=[agout[:]],
                )

                # ---- per 128-node tile ----
                for t in range(TILES):
                    hslab = hin[:, t * 128:(t + 1) * 128]
                    itile = sp.tile([128, 96], I16, tag="itile")
                    nc.sync.dma_start(out=itile[:], in_=idx[t])
                    mt = sp.tile([128, DEG], F16, tag="mt")
                    nc.sync.dma_start(out=mt[:], in_=msk[t])
                    pairt = gp.tile([128, DEG, 2 * D], F16, tag="pair")
                    for g in range(2):
                        nc.gpsimd.dma_gather(
                            out_ap=pairt[:, 6 * g:6 * g + 6, :], in_ap=agout[:],
                            idxs_ap=itile[:, 48 * g:48 * g + 48],
                            num_idxs=768, num_idxs_reg=768, elem_size=2 * D,
                            queue_num=(2 * t + g) % NQ,
                        )

                    # C = A (node-major); bpre is folded into bout on the host
                    apsum = ps.tile([128, D], F32, space="PSUM", tag="apsum")
                    nc.tensor.matmul(out=apsum[:], lhsT=hslab, rhs=w1t, start=True, stop=True)
                    C = sp.tile([128, D], F32, tag="C")
                    nc.scalar.activation(C[:], apsum[:], AF.Copy)

                    # select even/odd pair half per edge: gmh = even + mask*(odd-even)
                    gmh = gp.tile([128, DEG, D], F16, tag="gmh")
                    ev = pairt[:, :, 0:D]
                    od = pairt[:, :, D:2 * D]
                    mb3 = mt[:].unsqueeze(2).broadcast_to([128, DEG, D])
                    nc.vector.tensor_tensor(out=gmh[:], in0=od, in1=ev, op=A.subtract)
                    nc.vector.tensor_tensor(out=gmh[:], in0=gmh[:], in1=mb3, op=A.mult)
                    nc.vector.tensor_tensor(out=gmh[:], in0=gmh[:], in1=ev, op=A.add)

                    g2 = gp.tile([128, DEG, D], F16, tag="g2")
                    nc.scalar.square(g2[:], gmh[:])
                    wS = wp.tile([128, 6, D], F16, tag="wS")
                    wX = wp.tile([128, 6, D], F16, tag="wX")
                    wN = wp.tile([128, 6, D], F16, tag="wN")
                    w2b = wp.tile([128, 6, D], F16, tag="w2b")
                    S = sp.tile([128, D], F32, tag="S")
                    MX = sp.tile([128, D], F32, tag="MX")
                    MN = sp.tile([128, D], F32, tag="MN")
                    S2 = sp.tile([128, D], F32, tag="S2")
                    _tree(nc, nc.vector, gmh[:], wS, S[:], "add")
                    _tree(nc, nc.vector, gmh[:], wX, MX[:], "max")
                    _tree(nc, nc.vector, gmh[:], wN, MN[:], "min")
                    _tree(nc, nc.vector, g2[:], w2b, S2[:], "add")

                    meanB = sp.tile([128, D], F32, tag="meanB")
                    nc.vector.tensor_scalar_mul(meanB[:], S[:], 1.0 / DEG)
                    mean = sp.tile([128, D], F32, tag="mean")
                    nc.vector.tensor_tensor(out=mean[:], in0=meanB[:], in1=C[:], op=A.add)
                    mxc = sp.tile([128, D], F32, tag="mxc")
                    nc.vector.tensor_tensor(out=mxc[:], in0=MX[:], in1=C[:], op=A.add)
                    mnc = sp.tile([128, D], F32, tag="mnc")
                    nc.vector.tensor_tensor(out=mnc[:], in0=MN[:], in1=C[:], op=A.add)
                    m2 = sp.tile([128, D], F32, tag="m2")
                    nc.vector.tensor_tensor(out=m2[:], in0=meanB[:], in1=meanB[:], op=A.mult)
                    var = sp.tile([128, D], F32, tag="var")
                    nc.vector.scalar_tensor_tensor(
                        out=var[:], in0=S2[:], scalar=1.0 / DEG, in1=m2[:],
                        op0=A.mult, op1=A.subtract,
                    )
                    varc = sp.tile([128, D], F32, tag="varc")
                    nc.vector.tensor_scalar_max(varc[:], var[:], 0.0)
                    std = sp.tile([128, D], F32, tag="std")
                    nc.scalar.activation(std[:], varc[:], AF.Sqrt, bias=eps[:])

                    # transpose agg parts to feature-major and post-GEMM
                    hp = ps.tile([128, 128], F32, space="PSUM", tag="hp")
                    nc.tensor.matmul(out=hp[:], lhsT=whh, rhs=hslab, start=True, stop=False)
                    for j, part in enumerate([mean, mxc, mnc, std]):
                        ptp = ps.tile([128, 128], F32, space="PSUM", tag="tp")
                        nc.tensor.transpose(out=ptp[:], in_=part[:], identity=ident[:])
                        pts = sp.tile([128, 128], F32, tag="pts")
                        if j % 2 == 0:
                            nc.scalar.activation(pts[:], ptp[:], AF.Copy)
                        else:
                            nc.vector.tensor_copy(pts[:], ptp[:])
                        nc.tensor.matmul(
                            out=hp[:], lhsT=weff[j], rhs=pts[:],
                            start=False, stop=(j == 3),
                        )
                    nc.scalar.activation(
                        hout[:, t * 128:(t + 1) * 128], hp[:], AF.Identity, bias=bout,
                    )
                if cfg.NC < NP:
                    nc.vector.memset(hout[:, cfg.NC:NP], 0.0)

            nc.sync.dma_start(out=outT[:], in_=hT[(repeat * L) % 2][:])
    nc.compile()
    return nc


def prep_inputs(cfg, x, edge_index, Wpre, bpre, Wpost, bpost, Wlin, blin):
    x = np.asarray(x, np.float32)
    ei = np.asarray(edge_index)
    Wpre = np.asarray(Wpre, np.float32)
    bpre = np.asarray(bpre, np.float32)
    Wpost = np.asarray(Wpost, np.float32)
    bpost = np.asarray(bpost, np.float32)
    Wlin = np.asarray(Wlin, np.float32)
    blin = np.asarray(blin, np.float32)
    N, NC, NP, TILES = cfg.N, cfg.NC, cfg.NP, cfg.TILES

    row = ei[0].astype(np.int64)
    col = ei[1].astype(np.int64)
    assert (row == np.repeat(np.arange(N), DEG)).all(), "kernel assumes sorted rows, uniform degree"
    dlog = math.log(DEG + 1.0)
    k1 = dlog / AVG_LOG
    k2 = AVG_LOG / dlog

    wpack = np.zeros((D, L, 7, D), np.float32)
    bpack = np.zeros((D, L, 2), np.float32)
    I = np.eye(D, dtype=np.float32)
    for l in range(L):
        W1 = Wpre[l][:, :D]
        W2 = Wpre[l][:, D:]
        Wh = Wpost[l][:, :D]
        Wid = Wpost[l][:, D:5 * D]
        Wamp = Wpost[l][:, 5 * D:9 * D]
        Watt = Wpost[l][:, 9 * D:13 * D]
        Weff = Wlin[l] @ (Wid + k1 * Wamp + k2 * Watt)
        Whh = Wlin[l] @ Wh + I
        wpack[:, l, 0, :] = W1.T
        wpack[:, l, 1, :] = W2.T
        wpack[:, l, 2, :] = Whh.T
        for j in range(4):
            wpack[:, l, 3 + j, :] = Weff[:, j * D:(j + 1) * D].T
        bpack[:, l, 0] = bpre[l]
        bpack[:, l, 1] = (Wlin[l] @ bpost[l] + blin[l]
                          + (Weff[:, :D] + Weff[:, D:2*D] + Weff[:, 2*D:3*D]) @ bpre[l])

    in_maps = []
    for c in range(CORES):
        xs = x[c * NC:(c + 1) * NC]
        xT = np.zeros((D, NP), np.float32)
        xT[:, :NC] = xs.T
        cols = col[c * NC * DEG:(c + 1) * NC * DEG]
        cols = np.concatenate([cols, np.zeros(((NP - NC) * DEG,), np.int64)])
        cols = cols.reshape(NP, DEG)
        gr = (cols // NC) * NP + (cols % NC)  # global row in the AllGather table
        pr = (gr // 2).astype(np.int16).reshape(TILES, 128, DEG)
        par = (gr % 2).astype(np.float16).reshape(TILES, 128, DEG)
        # dma_gather idx layout: 2 gathers of 768 (slots 0-5 / 6-11); linear
        # position i = k_local*128 + p lives at [i%16, i//16], and the 16-row
        # block is replicated across all 8 GPSIMD cores' partition windows.
        idxa = np.zeros((TILES, 128, 96), np.int16)
        for g in range(2):
            sub = pr[:, :, 6 * g:6 * g + 6]             # (T, 128, 6)
            v = sub.transpose(0, 2, 1).reshape(TILES, 768)
            block = v.reshape(TILES, 48, 16).transpose(0, 2, 1)  # (T, 16, 48)
            idxa[:, :, 48 * g:48 * g + 48] = np.tile(block, (1, 8, 1))
        in_maps.append({
            "xT": xT,
            "idx": idxa,
            "msk": par,
            "wpack": wpack,
            "bpack": bpack,
        })
    return in_maps


_CACHE = {}


def kernel(x, edge_index, Wpre, bpre, Wpost, bpost, Wlin, blin):
    cfg = Cfg(np.asarray(x).shape[0])
    in_maps = prep_inputs(cfg, x, edge_index, Wpre, bpre, Wpost, bpost, Wlin, blin)
    if cfg.N not in _CACHE:
        _CACHE[cfg.N] = build(cfg)
    nc = _CACHE[cfg.N]
    res = run_bass_kernel_spmd(nc, in_maps, list(range(CORES)))
    outs = []
    for c in range(CORES):
        oT = res.results[c]["outT"]
        outs.append(np.ascontiguousarray(oT[:, :cfg.NC].T))
    return np.concatenate(outs, axis=0).astype(np.float32)
